# revision 18
# baseline (speedup 1.0000x reference)
"""Trainium2 Bass kernel for RoPE + GQA causal attention (B=1, S=2048, HID=2048,
NH=16, NKV=4, HD=128), tensor-parallel over heads across 8 NeuronCores.

Sharding: core c computes q heads {2c, 2c+1} and kv head c//2, plus the
corresponding slice of the output projection (wo input-dim shard). Each core
emits a partial [S, HID] output; the host sums the 8 partials (the unshard
step for an input-dim-sharded matmul).

Per-core dataflow (all "transposed" layout, d on partitions; matmul path in
bf16 with f32 PSUM accumulation):
  xT[h,s] -> QT/KT/VT = W^T-slices @ xT          (PSUM accum over 16 h-chunks)
  RoPE(qT) = C1 * (R1 @ qT) + C2 * (R2 @ qT)     (R1/R2 are 128x128 sign-perms,
                                                  C1/C2 built from cos/sin)
  V transposed to [s,d] tiles via PE transpose
  scoresT[sk,sq] = KT^T-tile @ QT-chunk
  expT = exp(scale * scoresT + causal mask)      (ACT, bf16 out)
  attnT[d,sq] += V-tile @ expT                   (PSUM accum over sk tiles)
  den[1,sq]  += ones^T @ expT                    (PSUM accum)
  attnT = attnT / broadcast(den)                 (gpsimd bcast + DVE divide)
  partial[s,h] = attnT-slices^T @ woT            (natural layout out, f32)
"""

import os
import sys
from contextlib import ExitStack

for _p in ("/opt/trn_rl_repo", "/root/.axon_site/_ro/trn_rl_repo"):
    if os.path.isdir(_p) and _p not in sys.path:
        sys.path.append(_p)

import ml_dtypes
import numpy as np

import concourse.bass as bass
import concourse.mybir as mybir
import concourse.tile as tile
from concourse import bacc, bass_utils
from concourse.masks import make_identity

S, HID, NH, NKV, HD = 2048, 2048, 16, 4, 128
HH = HD // 2  # 64
NCORES = 8
QH_PER_CORE = NH // NCORES  # 2
SCALE = float(1.0 / np.sqrt(HD))
MASK_VAL = -30000.0

F32 = mybir.dt.float32
BF16 = mybir.dt.bfloat16
NPBF = ml_dtypes.bfloat16

SC = 512          # s-chunk width (free dim of most matmuls)
NSC = S // SC     # 4
NKC = HID // 128  # 16 contraction chunks
NST = S // 128    # 16 s-tiles


def build_graph():
    nc = bacc.Bacc(trn_type="TRN2", enable_partition_id=False)

    xT = nc.dram_tensor("xt", [HID, S], BF16, kind="ExternalInput")
    wqkvT = nc.dram_tensor("wqkvt", [HID, 4 * HD], BF16, kind="ExternalInput")
    woT = nc.dram_tensor("wot", [QH_PER_CORE * HD, HID], BF16, kind="ExternalInput")
    c1d = nc.dram_tensor("c1", [HD, S], F32, kind="ExternalInput")
    c2d = nc.dram_tensor("c2", [HD, S], F32, kind="ExternalInput")
    r1d = nc.dram_tensor("r1t", [HD, HD], BF16, kind="ExternalInput")
    r2d = nc.dram_tensor("r2t", [HD, HD], BF16, kind="ExternalInput")
    outd = nc.dram_tensor("out", [S, HID], F32, kind="ExternalOutput")

    xT_t = xT.rearrange("(ko p) s -> p ko s", p=128)       # [128, 16, 2048]
    wqkv_t = wqkvT.rearrange("(ko p) o -> p ko o", p=128)  # [128, 16, 512]
    wo_t = woT.rearrange("(g p) h -> p g h", p=128)        # [128, 2, 2048]

    with tile.TileContext(nc) as tc, ExitStack() as ctx:
        # ---- pools: one shared PSUM set, exactly 8 banks ------------------
        consts = ctx.enter_context(tc.tile_pool(name="consts", bufs=1))
        persist = ctx.enter_context(tc.tile_pool(name="persist", bufs=1))
        work = ctx.enter_context(tc.tile_pool(name="work", bufs=2))
        expp = ctx.enter_context(tc.tile_pool(name="expp", bufs=4))
        outp = ctx.enter_context(tc.tile_pool(name="outp", bufs=4))
        nrm = ctx.enter_context(tc.tile_pool(name="nrm", bufs=2))

        ps_mm = ctx.enter_context(tc.tile_pool(name="ps_mm", bufs=3, space="PSUM"))
        ps_rv = ctx.enter_context(tc.tile_pool(name="ps_rv", bufs=2, space="PSUM"))
        ps_acc = ctx.enter_context(tc.tile_pool(name="ps_acc", bufs=2, space="PSUM"))
        ps_den = ctx.enter_context(tc.tile_pool(name="ps_den", bufs=1, space="PSUM"))

        # ---- persistent activations ---------------------------------------
        qTs = [persist.tile([128, S], BF16, tag=f"qT{h}", name=f"qT{h}")
               for h in range(QH_PER_CORE)]
        kT = persist.tile([128, S], BF16, tag="kT")
        v_sd = persist.tile([128, NST, HD], BF16, tag="v_sd")
        aoTs = [persist.tile([128, S], BF16, tag=f"aoT{h}", name=f"aoT{h}")
                for h in range(QH_PER_CORE)]

        # ---- input DMAs: first chunk's inputs lead the queues -------------
        wqkv_sb = consts.tile([128, NKC, 512], BF16)
        xt0 = work.tile([128, NKC, SC], BF16, tag="xt", name="xt0")
        for kc in range(NKC):
            eng = nc.sync if kc % 2 == 0 else nc.gpsimd
            eng.dma_start(wqkv_sb[:, kc, :], wqkv_t[:, kc, :])
            eng2 = nc.gpsimd if kc % 2 == 0 else nc.sync
            eng2.dma_start(xt0[:, kc, :], xT_t[:, kc, 0:SC])
        c1_sb = consts.tile([128, S], F32)
        c2_sb = consts.tile([128, S], F32)
        nc.sync.dma_start(c1_sb, c1d[:, :])
        nc.sync.dma_start(c2_sb, c2d[:, :])

        # ---- constants -----------------------------------------------------
        ident = consts.tile([128, 128], BF16)
        make_identity(nc, ident)

        # multiplicative causal mask for the diagonal 128x128 block of a
        # scoresT exp-tile: 1 where sq >= sk i.e. col >= row, else 0
        trimask = consts.tile([128, 128], BF16)
        nc.gpsimd.memset(trimask, 1.0)
        nc.gpsimd.affine_select(
            out=trimask,
            in_=trimask,
            compare_op=mybir.AluOpType.is_ge,
            fill=0.0,
            base=0,
            pattern=[[1, 128]],       # + 1*col
            channel_multiplier=-1,    # - row
        )

        ones_col = consts.tile([128, 1], BF16)
        nc.vector.memset(ones_col, 1.0)

        r1_sb = consts.tile([128, 128], BF16)
        r2_sb = consts.tile([128, 128], BF16)
        nc.sync.dma_start(r1_sb, r1d[:, :])
        nc.sync.dma_start(r2_sb, r2d[:, :])
        wo_sb = consts.tile([128, QH_PER_CORE, HID], BF16)
        for g in range(QH_PER_CORE):
            for hc in range(NSC):
                nc.sync.dma_start(
                    wo_sb[:, g, hc * SC:(hc + 1) * SC],
                    wo_t[:, g, hc * SC:(hc + 1) * SC],
                )

        def rope_into(dst_chunk, psum_raw, j):
            """dst = C1*(R1@raw) + C2*(R2@raw), raw read from PSUM."""
            raw = work.tile([128, SC], BF16, tag="rope_raw", name="rope_raw")
            nc.vector.tensor_copy(out=raw, in_=psum_raw)
            ps_u = ps_rv.tile([128, SC], F32, tag="rv", name="ps_u")
            ps_w = ps_rv.tile([128, SC], F32, tag="rv", name="ps_w")
            nc.tensor.matmul(ps_u, r1_sb, raw, start=True, stop=True)
            nc.tensor.matmul(ps_w, r2_sb, raw, start=True, stop=True)
            csl = slice(j * SC, (j + 1) * SC)
            t1 = work.tile([128, SC], F32, tag="rope_t1", name="t1")
            t2 = work.tile([128, SC], F32, tag="rope_t2", name="t2")
            nc.vector.tensor_mul(out=t1, in0=ps_u, in1=c1_sb[:, csl])
            nc.vector.tensor_mul(out=t2, in0=ps_w, in1=c2_sb[:, csl])
            nc.vector.tensor_add(out=dst_chunk, in0=t1, in1=t2)

        def proj_chunk(j):
            csl = slice(j * SC, (j + 1) * SC)
            if j == 0:
                xt = xt0
            else:
                xt = work.tile([128, NKC, SC], BF16, tag="xt", name="xt")
                for kc in range(NKC):
                    nc.sync.dma_start(xt[:, kc, :], xT_t[:, kc, csl])

            for t in range(4):  # q0, q1, k, v (columns of wqkv)
                osl = slice(t * 128, (t + 1) * 128)
                ps_p = ps_mm.tile([128, SC], F32, tag="mm", name="ps_p")
                for kc in range(NKC):
                    nc.tensor.matmul(
                        ps_p,
                        wqkv_sb[:, kc, osl],
                        xt[:, kc, :],
                        start=(kc == 0),
                        stop=(kc == NKC - 1),
                    )
                if t < 2:
                    rope_into(qTs[t][:, csl], ps_p, j)
                elif t == 2:
                    rope_into(kT[:, csl], ps_p, j)
                else:
                    # V^T chunk -> bf16 -> PE-transpose into [s,d] tiles
                    vt_raw = work.tile([128, SC], BF16, tag="vt_raw",
                                       name="vt_raw")
                    nc.vector.tensor_copy(out=vt_raw, in_=ps_p)
                    for b in range(SC // 128):
                        st = j * (SC // 128) + b
                        ps_t = ps_rv.tile([128, 128], BF16, tag="rv",
                                          name="ps_t")
                        nc.tensor.transpose(
                            ps_t, vt_raw[:, b * 128:(b + 1) * 128], ident
                        )
                        nc.vector.tensor_copy(out=v_sd[:, st, :], in_=ps_t)

        def attention_chunk(j):
            csl = slice(j * SC, (j + 1) * SC)
            for h in range(QH_PER_CORE):
                qc = qTs[h][:, csl]
                nk = 4 * j + 4  # causal: sk tiles 0..4j+3
                ps_o = ps_acc.tile([128, SC], F32, tag="attn", name="ps_o")
                ps_d = ps_den.tile([1, SC], F32, tag="den", name="ps_d")
                for k in range(nk):
                    ps_s = ps_mm.tile([128, SC], F32, tag="mm", name="ps_s")
                    nc.tensor.matmul(
                        ps_s, kT[:, k * 128:(k + 1) * 128], qc,
                        start=True, stop=True,
                    )
                    m = k - 4 * j
                    e = expp.tile([128, SC], BF16, tag="exp", name="e")
                    if m >= 0:
                        # diagonal block: exp valid cols, then 0/1-mask the
                        # triangular middle on gpsimd (keeps DVE/ACT paths
                        # clear); cols < 128m are fully masked -> memset 0
                        dsl = slice(m * 128, (m + 1) * 128)
                        if m > 0:
                            nc.gpsimd.memset(e[:, : m * 128], 0.0)
                        nc.scalar.activation(
                            out=e[:, m * 128:], in_=ps_s[:, m * 128:],
                            func=mybir.ActivationFunctionType.Exp,
                            scale=SCALE,
                        )
                        nc.gpsimd.tensor_mul(
                            out=e[:, dsl], in0=e[:, dsl], in1=trimask
                        )
                    else:
                        nc.scalar.activation(
                            out=e, in_=ps_s,
                            func=mybir.ActivationFunctionType.Exp,
                            scale=SCALE,
                        )
                    nc.tensor.matmul(
                        ps_o, v_sd[:, k, :], e,
                        start=(k == 0), stop=(k == nk - 1),
                    )
                    nc.tensor.matmul(
                        ps_d, ones_col, e,
                        start=(k == 0), stop=(k == nk - 1),
                    )
                # normalize: attnT[:, sq] *= 1/den[sq]; cheap DVE copy
                # releases the den PSUM bank, reciprocal runs off-bank
                den_sb = nrm.tile([1, SC], F32, tag="den_sb", name="den_sb")
                nc.vector.tensor_copy(out=den_sb, in_=ps_d)
                recip = nrm.tile([1, SC], F32, tag="recip", name="recip")
                nc.vector.reciprocal(out=recip, in_=den_sb)
                db = nrm.tile([128, SC], F32, tag="den_b", name="db")
                nc.gpsimd.partition_broadcast(db, recip[:1, :])
                nc.vector.tensor_mul(out=aoTs[h][:, csl], in0=ps_o, in1=db)

        def outproj_chunk(j):
            for b in range(SC // 128):
                st = j * (SC // 128) + b
                ssl = slice(st * 128, (st + 1) * 128)
                for hc in range(NSC):
                    hsl = slice(hc * SC, (hc + 1) * SC)
                    ps_c = ps_mm.tile([128, SC], F32, tag="mm", name="ps_c")
                    for h in range(QH_PER_CORE):
                        nc.tensor.matmul(
                            ps_c,
                            aoTs[h][:, ssl],
                            wo_sb[:, h, hsl],
                            start=(h == 0),
                            stop=(h == QH_PER_CORE - 1),
                        )
                    ob = outp.tile([128, SC], F32, tag="outb", name="ob")
                    if hc % 2 == 0:
                        nc.vector.tensor_copy(out=ob, in_=ps_c)
                    else:
                        nc.scalar.copy(out=ob, in_=ps_c)
                    eng = nc.sync if (b + hc) % 2 == 0 else nc.gpsimd
                    eng.dma_start(outd[ssl, hsl], ob)

        for j in range(NSC):
            proj_chunk(j)
        attention_chunk(0)
        for j in range(1, NSC):
            attention_chunk(j)
            outproj_chunk(j - 1)
        outproj_chunk(NSC - 1)

    nc.finalize()
    return nc


def shard_inputs(x, cos, sin, wq, wk, wv, wo):
    x = np.asarray(x, np.float32).reshape(S, HID)
    cos = np.asarray(cos, np.float32)
    sin = np.asarray(sin, np.float32)
    wq = np.asarray(wq, np.float32)
    wk = np.asarray(wk, np.float32)
    wv = np.asarray(wv, np.float32)
    wo = np.asarray(wo, np.float32)

    xT = np.ascontiguousarray(x.T).astype(NPBF)

    cos_h, sin_h = cos[:, :HH].T, sin[:, :HH].T       # [64, S]
    c1 = np.ascontiguousarray(np.concatenate([cos_h, -sin_h], axis=0))
    c2 = np.ascontiguousarray(np.concatenate([sin_h, cos_h], axis=0))

    r1 = np.zeros((HD, HD), np.float32)
    for i in range(HH // 2):
        r1[2 * i, 2 * i + 1] = -1.0
        r1[2 * i + 1, 2 * i] = 1.0
    r1[HH:, :] = r1[:HH, :]
    r2 = np.zeros((HD, HD), np.float32)
    for d in range(HH):
        r2[d, d + HH] = 1.0
        r2[d + HH, d + HH] = 1.0
    r1t = np.ascontiguousarray(r1.T).astype(NPBF)  # lhsT for out = R1 @ rhs
    r2t = np.ascontiguousarray(r2.T).astype(NPBF)

    in_maps = []
    for c in range(NCORES):
        h0 = QH_PER_CORE * c
        kvh = h0 * NKV // NH
        wq_c = wq[h0 * HD:(h0 + QH_PER_CORE) * HD, :]    # [256, HID]
        wk_c = wk[kvh * HD:(kvh + 1) * HD, :]            # [128, HID]
        wv_c = wv[kvh * HD:(kvh + 1) * HD, :]
        wqkvT_c = np.ascontiguousarray(
            np.concatenate([wq_c, wk_c, wv_c], axis=0).T).astype(NPBF)
        woT_c = np.ascontiguousarray(
            wo[:, h0 * HD:(h0 + QH_PER_CORE) * HD].T).astype(NPBF)
        in_maps.append({
            "xt": xT,
            "wqkvt": wqkvT_c,
            "wot": woT_c,
            "c1": c1,
            "c2": c2,
            "r1t": r1t,
            "r2t": r2t,
        })
    return in_maps


_CACHED_NC = None


def kernel(x, cos, sin, wq, wk, wv, wo, _trace=False, _tmpdir=None):
    global _CACHED_NC
    in_maps = shard_inputs(x, cos, sin, wq, wk, wv, wo)
    if _CACHED_NC is None:
        _CACHED_NC = build_graph()
    nc = _CACHED_NC
    res = bass_utils.run_bass_kernel_spmd(
        nc, in_maps, core_ids=list(range(NCORES)),
        trace=_trace, tmpdir=_tmpdir,
    )
    total = np.zeros((S, HID), np.float32)
    for r in res.results:
        total += r["out"]
    out = total.reshape(1, S, HID)
    if _trace:
        return out, res
    return out


# revision 19
# speedup vs baseline: 1.3264x; 1.3264x over previous
"""Trainium2 Bass kernel for RoPE + GQA causal attention (B=1, S=2048, HID=2048,
NH=16, NKV=4, HD=128), tensor-parallel over heads across 8 NeuronCores.

Sharding: core c computes q heads {2c, 2c+1} and kv head c//2, plus the
corresponding slice of the output projection (wo input-dim shard). Each core
emits a partial [S, HID] output; the host sums the 8 partials (the unshard
step for an input-dim-sharded matmul).

Per-core dataflow (all "transposed" layout, d on partitions; matmul path in
bf16 with f32 PSUM accumulation):
  xT[h,s] -> QT/KT/VT = W^T-slices @ xT          (PSUM accum over 16 h-chunks)
  RoPE(qT) = C1 * (R1 @ qT) + C2 * (R2 @ qT)     (R1/R2 are 128x128 sign-perms,
                                                  C1/C2 built from cos/sin)
  V transposed to [s,d] tiles via PE transpose
  scoresT[sk,sq] = KT^T-tile @ QT-chunk
  expT = exp(scale * scoresT + causal mask)      (ACT, bf16 out)
  attnT[d,sq] += V-tile @ expT                   (PSUM accum over sk tiles)
  den[1,sq]  += ones^T @ expT                    (PSUM accum)
  attnT = attnT / broadcast(den)                 (gpsimd bcast + DVE divide)
  partial[s,h] = attnT-slices^T @ woT            (natural layout out, f32)
"""

import os
import sys
from contextlib import ExitStack

for _p in ("/opt/trn_rl_repo", "/root/.axon_site/_ro/trn_rl_repo"):
    if os.path.isdir(_p) and _p not in sys.path:
        sys.path.append(_p)

import ml_dtypes
import numpy as np

import concourse.bass as bass
import concourse.mybir as mybir
import concourse.tile as tile
from concourse import bacc, bass_utils
from concourse.masks import make_identity

S, HID, NH, NKV, HD = 2048, 2048, 16, 4, 128
HH = HD // 2  # 64
NCORES = 8
QH_PER_CORE = NH // NCORES  # 2
SCALE = float(1.0 / np.sqrt(HD))
MASK_VAL = -30000.0

F32 = mybir.dt.float32
BF16 = mybir.dt.bfloat16
NPBF = ml_dtypes.bfloat16

SC = 512          # s-chunk width (free dim of most matmuls)
NSC = S // SC     # 4
NKC = HID // 128  # 16 contraction chunks
NST = S // 128    # 16 s-tiles


def build_graph():
    nc = bacc.Bacc(trn_type="TRN2", enable_partition_id=False)

    xT = nc.dram_tensor("xt", [HID, S], BF16, kind="ExternalInput")
    wqkvT = nc.dram_tensor("wqkvt", [HID, 4 * HD], BF16, kind="ExternalInput")
    woT = nc.dram_tensor("wot", [QH_PER_CORE * HD, HID], BF16, kind="ExternalInput")
    c1d = nc.dram_tensor("c1", [HD, S], F32, kind="ExternalInput")
    c2d = nc.dram_tensor("c2", [HD, S], F32, kind="ExternalInput")
    r1d = nc.dram_tensor("r1t", [HD, HD], BF16, kind="ExternalInput")
    r2d = nc.dram_tensor("r2t", [HD, HD], BF16, kind="ExternalInput")
    outd = nc.dram_tensor("out", [S, HID], F32, kind="ExternalOutput")

    xT_t = xT.rearrange("(ko p) s -> p ko s", p=128)       # [128, 16, 2048]
    wqkv_t = wqkvT.rearrange("(ko p) o -> p ko o", p=128)  # [128, 16, 512]
    wo_t = woT.rearrange("(g p) h -> p g h", p=128)        # [128, 2, 2048]

    with tile.TileContext(nc) as tc, ExitStack() as ctx:
        # ---- pools: one shared PSUM set, exactly 8 banks ------------------
        consts = ctx.enter_context(tc.tile_pool(name="consts", bufs=1))
        persist = ctx.enter_context(tc.tile_pool(name="persist", bufs=1))
        work = ctx.enter_context(tc.tile_pool(name="work", bufs=2))
        expp = ctx.enter_context(tc.tile_pool(name="expp", bufs=4))
        outp = ctx.enter_context(tc.tile_pool(name="outp", bufs=4))
        nrm = ctx.enter_context(tc.tile_pool(name="nrm", bufs=2))

        ps_mm = ctx.enter_context(tc.tile_pool(name="ps_mm", bufs=3, space="PSUM"))
        ps_rv = ctx.enter_context(tc.tile_pool(name="ps_rv", bufs=2, space="PSUM"))
        ps_acc = ctx.enter_context(tc.tile_pool(name="ps_acc", bufs=2, space="PSUM"))
        ps_den = ctx.enter_context(tc.tile_pool(name="ps_den", bufs=1, space="PSUM"))

        # ---- persistent activations ---------------------------------------
        qTs = [persist.tile([128, S], BF16, tag=f"qT{h}", name=f"qT{h}")
               for h in range(QH_PER_CORE)]
        kT = persist.tile([128, S], BF16, tag="kT")
        v_sd = persist.tile([128, NST, HD], BF16, tag="v_sd")
        aoTs = [persist.tile([128, S], BF16, tag=f"aoT{h}", name=f"aoT{h}")
                for h in range(QH_PER_CORE)]

        # ---- input DMAs: first chunk's inputs lead the queues -------------
        wqkv_sb = consts.tile([128, NKC, 512], BF16)
        xt0 = work.tile([128, NKC, SC], BF16, tag="xt", name="xt0")
        for kc in range(NKC):
            eng = nc.sync if kc % 2 == 0 else nc.gpsimd
            eng.dma_start(wqkv_sb[:, kc, :], wqkv_t[:, kc, :])
            eng2 = nc.gpsimd if kc % 2 == 0 else nc.sync
            eng2.dma_start(xt0[:, kc, :], xT_t[:, kc, 0:SC])
        c1_sb = consts.tile([128, S], F32)
        c2_sb = consts.tile([128, S], F32)
        nc.sync.dma_start(c1_sb, c1d[:, :])
        nc.sync.dma_start(c2_sb, c2d[:, :])

        # ---- constants -----------------------------------------------------
        ident = consts.tile([128, 128], BF16)
        make_identity(nc, ident)

        # additive causal mask for the diagonal 128x128 block of a scoresT
        # tile: keep (0) where sq >= sk i.e. col >= row, else MASK_VAL
        trimask = consts.tile([128, 128], F32)
        nc.gpsimd.memset(trimask, 0.0)
        nc.gpsimd.affine_select(
            out=trimask,
            in_=trimask,
            compare_op=mybir.AluOpType.is_ge,
            fill=MASK_VAL,
            base=0,
            pattern=[[1, 128]],       # + 1*col
            channel_multiplier=-1,    # - row
        )

        ones_col = consts.tile([128, 1], BF16)
        nc.vector.memset(ones_col, 1.0)

        r1_sb = consts.tile([128, 128], BF16)
        r2_sb = consts.tile([128, 128], BF16)
        nc.sync.dma_start(r1_sb, r1d[:, :])
        nc.sync.dma_start(r2_sb, r2d[:, :])
        wo_sb = consts.tile([128, QH_PER_CORE, HID], BF16)
        for g in range(QH_PER_CORE):
            for hc in range(NSC):
                nc.sync.dma_start(
                    wo_sb[:, g, hc * SC:(hc + 1) * SC],
                    wo_t[:, g, hc * SC:(hc + 1) * SC],
                )

        def rope_into(dst_chunk, psum_raw, j):
            """dst = C1*(R1@raw) + C2*(R2@raw), raw read from PSUM."""
            raw = work.tile([128, SC], BF16, tag="rope_raw", name="rope_raw")
            nc.vector.tensor_copy(out=raw, in_=psum_raw)
            ps_u = ps_rv.tile([128, SC], F32, tag="rv", name="ps_u")
            ps_w = ps_rv.tile([128, SC], F32, tag="rv", name="ps_w")
            nc.tensor.matmul(ps_u, r1_sb, raw, start=True, stop=True)
            nc.tensor.matmul(ps_w, r2_sb, raw, start=True, stop=True)
            csl = slice(j * SC, (j + 1) * SC)
            t1 = work.tile([128, SC], F32, tag="rope_t1", name="t1")
            t2 = work.tile([128, SC], F32, tag="rope_t2", name="t2")
            nc.vector.tensor_mul(out=t1, in0=ps_u, in1=c1_sb[:, csl])
            nc.vector.tensor_mul(out=t2, in0=ps_w, in1=c2_sb[:, csl])
            nc.vector.tensor_add(out=dst_chunk, in0=t1, in1=t2)

        def proj_chunk(j):
            csl = slice(j * SC, (j + 1) * SC)
            if j == 0:
                xt = xt0
            else:
                xt = work.tile([128, NKC, SC], BF16, tag="xt", name="xt")
                for kc in range(NKC):
                    nc.sync.dma_start(xt[:, kc, :], xT_t[:, kc, csl])

            for t in range(4):  # q0, q1, k, v (columns of wqkv)
                osl = slice(t * 128, (t + 1) * 128)
                ps_p = ps_mm.tile([128, SC], F32, tag="mm", name="ps_p")
                for kc in range(NKC):
                    nc.tensor.matmul(
                        ps_p,
                        wqkv_sb[:, kc, osl],
                        xt[:, kc, :],
                        start=(kc == 0),
                        stop=(kc == NKC - 1),
                    )
                if t < 2:
                    rope_into(qTs[t][:, csl], ps_p, j)
                elif t == 2:
                    rope_into(kT[:, csl], ps_p, j)
                else:
                    # V^T chunk -> bf16 -> PE-transpose into [s,d] tiles
                    vt_raw = work.tile([128, SC], BF16, tag="vt_raw",
                                       name="vt_raw")
                    nc.vector.tensor_copy(out=vt_raw, in_=ps_p)
                    for b in range(SC // 128):
                        st = j * (SC // 128) + b
                        ps_t = ps_rv.tile([128, 128], BF16, tag="rv",
                                          name="ps_t")
                        nc.tensor.transpose(
                            ps_t, vt_raw[:, b * 128:(b + 1) * 128], ident
                        )
                        nc.vector.tensor_copy(out=v_sd[:, st, :], in_=ps_t)

        def attention_chunk(j):
            csl = slice(j * SC, (j + 1) * SC)
            for h in range(QH_PER_CORE):
                qc = qTs[h][:, csl]
                nk = 4 * j + 4  # causal: sk tiles 0..4j+3
                ps_o = ps_acc.tile([128, SC], F32, tag="attn", name="ps_o")
                ps_d = ps_den.tile([1, SC], F32, tag="den", name="ps_d")
                for k in range(nk):
                    ps_s = ps_mm.tile([128, SC], F32, tag="mm", name="ps_s")
                    nc.tensor.matmul(
                        ps_s, kT[:, k * 128:(k + 1) * 128], qc,
                        start=True, stop=True,
                    )
                    m = k - 4 * j
                    e = expp.tile([128, SC], BF16, tag="exp", name="e")
                    if m >= 0:
                        # diagonal block: triangular mask on cols
                        # [128m, 128m+128); cols < 128m fully masked
                        dsl = slice(m * 128, (m + 1) * 128)
                        nc.vector.tensor_add(
                            out=ps_s[:, dsl], in0=ps_s[:, dsl], in1=trimask
                        )
                        if m > 0:
                            nc.gpsimd.memset(e[:, : m * 128], 0.0)
                        nc.scalar.activation(
                            out=e[:, m * 128:], in_=ps_s[:, m * 128:],
                            func=mybir.ActivationFunctionType.Exp,
                            scale=SCALE,
                        )
                    else:
                        nc.scalar.activation(
                            out=e, in_=ps_s,
                            func=mybir.ActivationFunctionType.Exp,
                            scale=SCALE,
                        )
                    nc.tensor.matmul(
                        ps_o, v_sd[:, k, :], e,
                        start=(k == 0), stop=(k == nk - 1),
                    )
                    nc.tensor.matmul(
                        ps_d, ones_col, e,
                        start=(k == 0), stop=(k == nk - 1),
                    )
                # normalize: attnT[:, sq] *= 1/den[sq]; cheap DVE copy
                # releases the den PSUM bank, reciprocal runs off-bank
                den_sb = nrm.tile([1, SC], F32, tag="den_sb", name="den_sb")
                nc.vector.tensor_copy(out=den_sb, in_=ps_d)
                recip = nrm.tile([1, SC], F32, tag="recip", name="recip")
                nc.vector.reciprocal(out=recip, in_=den_sb)
                db = nrm.tile([128, SC], F32, tag="den_b", name="db")
                nc.gpsimd.partition_broadcast(db, recip[:1, :])
                nc.vector.tensor_mul(out=aoTs[h][:, csl], in0=ps_o, in1=db)

        def outproj_chunk(j):
            for b in range(SC // 128):
                st = j * (SC // 128) + b
                ssl = slice(st * 128, (st + 1) * 128)
                for hc in range(NSC):
                    hsl = slice(hc * SC, (hc + 1) * SC)
                    ps_c = ps_mm.tile([128, SC], F32, tag="mm", name="ps_c")
                    for h in range(QH_PER_CORE):
                        nc.tensor.matmul(
                            ps_c,
                            aoTs[h][:, ssl],
                            wo_sb[:, h, hsl],
                            start=(h == 0),
                            stop=(h == QH_PER_CORE - 1),
                        )
                    ob = outp.tile([128, SC], F32, tag="outb", name="ob")
                    if hc % 2 == 0:
                        nc.vector.tensor_copy(out=ob, in_=ps_c)
                    else:
                        nc.scalar.copy(out=ob, in_=ps_c)
                    eng = nc.sync if (b + hc) % 2 == 0 else nc.gpsimd
                    eng.dma_start(outd[ssl, hsl], ob)

        for j in range(NSC):
            proj_chunk(j)
        attention_chunk(0)
        for j in range(1, NSC):
            attention_chunk(j)
            outproj_chunk(j - 1)
        outproj_chunk(NSC - 1)

    nc.finalize()
    return nc


def shard_inputs(x, cos, sin, wq, wk, wv, wo):
    x = np.asarray(x, np.float32).reshape(S, HID)
    cos = np.asarray(cos, np.float32)
    sin = np.asarray(sin, np.float32)
    wq = np.asarray(wq, np.float32)
    wk = np.asarray(wk, np.float32)
    wv = np.asarray(wv, np.float32)
    wo = np.asarray(wo, np.float32)

    xT = np.ascontiguousarray(x.T).astype(NPBF)

    cos_h, sin_h = cos[:, :HH].T, sin[:, :HH].T       # [64, S]
    c1 = np.ascontiguousarray(np.concatenate([cos_h, -sin_h], axis=0))
    c2 = np.ascontiguousarray(np.concatenate([sin_h, cos_h], axis=0))

    r1 = np.zeros((HD, HD), np.float32)
    for i in range(HH // 2):
        r1[2 * i, 2 * i + 1] = -1.0
        r1[2 * i + 1, 2 * i] = 1.0
    r1[HH:, :] = r1[:HH, :]
    r2 = np.zeros((HD, HD), np.float32)
    for d in range(HH):
        r2[d, d + HH] = 1.0
        r2[d + HH, d + HH] = 1.0
    r1t = np.ascontiguousarray(r1.T).astype(NPBF)  # lhsT for out = R1 @ rhs
    r2t = np.ascontiguousarray(r2.T).astype(NPBF)

    in_maps = []
    for c in range(NCORES):
        h0 = QH_PER_CORE * c
        kvh = h0 * NKV // NH
        wq_c = wq[h0 * HD:(h0 + QH_PER_CORE) * HD, :]    # [256, HID]
        wk_c = wk[kvh * HD:(kvh + 1) * HD, :]            # [128, HID]
        wv_c = wv[kvh * HD:(kvh + 1) * HD, :]
        wqkvT_c = np.ascontiguousarray(
            np.concatenate([wq_c, wk_c, wv_c], axis=0).T).astype(NPBF)
        woT_c = np.ascontiguousarray(
            wo[:, h0 * HD:(h0 + QH_PER_CORE) * HD].T).astype(NPBF)
        in_maps.append({
            "xt": xT,
            "wqkvt": wqkvT_c,
            "wot": woT_c,
            "c1": c1,
            "c2": c2,
            "r1t": r1t,
            "r2t": r2t,
        })
    return in_maps


_CACHED_NC = None


def kernel(x, cos, sin, wq, wk, wv, wo, _trace=False, _tmpdir=None):
    global _CACHED_NC
    in_maps = shard_inputs(x, cos, sin, wq, wk, wv, wo)
    if _CACHED_NC is None:
        _CACHED_NC = build_graph()
    nc = _CACHED_NC
    res = bass_utils.run_bass_kernel_spmd(
        nc, in_maps, core_ids=list(range(NCORES)),
        trace=_trace, tmpdir=_tmpdir,
    )
    total = np.zeros((S, HID), np.float32)
    for r in res.results:
        total += r["out"]
    out = total.reshape(1, S, HID)
    if _trace:
        return out, res
    return out


# revision 20
# speedup vs baseline: 1.3455x; 1.0144x over previous
"""Trainium2 Bass kernel for RoPE + GQA causal attention (B=1, S=2048, HID=2048,
NH=16, NKV=4, HD=128), tensor-parallel over heads across 8 NeuronCores.

Sharding: core c computes q heads {2c, 2c+1} and kv head c//2, plus the
corresponding slice of the output projection (wo input-dim shard). Each core
emits a partial [S, HID] output; the host sums the 8 partials (the unshard
step for an input-dim-sharded matmul).

Per-core dataflow (all "transposed" layout, d on partitions; matmul path in
bf16 with f32 PSUM accumulation):
  xT[h,s] -> QT/KT/VT = W^T-slices @ xT          (PSUM accum over 16 h-chunks)
  RoPE(qT) = C1 * (R1 @ qT) + C2 * (R2 @ qT)     (R1/R2 are 128x128 sign-perms,
                                                  C1/C2 built from cos/sin)
  V transposed to [s,d] tiles via PE transpose
  scoresT[sk,sq] = KT^T-tile @ QT-chunk
  expT = exp(scale * scoresT + causal mask)      (ACT, bf16 out)
  attnT[d,sq] += V-tile @ expT                   (PSUM accum over sk tiles)
  den[1,sq]  += ones^T @ expT                    (PSUM accum)
  attnT = attnT / broadcast(den)                 (gpsimd bcast + DVE divide)
  partial[s,h] = attnT-slices^T @ woT            (natural layout out, f32)
"""

import os
import sys
from contextlib import ExitStack

for _p in ("/opt/trn_rl_repo", "/root/.axon_site/_ro/trn_rl_repo"):
    if os.path.isdir(_p) and _p not in sys.path:
        sys.path.append(_p)

import ml_dtypes
import numpy as np

import concourse.bass as bass
import concourse.mybir as mybir
import concourse.tile as tile
from concourse import bacc, bass_utils
from concourse.masks import make_identity

S, HID, NH, NKV, HD = 2048, 2048, 16, 4, 128
HH = HD // 2  # 64
NCORES = 8
QH_PER_CORE = NH // NCORES  # 2
SCALE = float(1.0 / np.sqrt(HD))
MASK_VAL = -30000.0

F32 = mybir.dt.float32
BF16 = mybir.dt.bfloat16
NPBF = ml_dtypes.bfloat16

SC = 512          # s-chunk width (free dim of most matmuls)
NSC = S // SC     # 4
NKC = HID // 128  # 16 contraction chunks
NST = S // 128    # 16 s-tiles


def build_graph():
    nc = bacc.Bacc(trn_type="TRN2", enable_partition_id=False)

    xT = nc.dram_tensor("xt", [HID, S], BF16, kind="ExternalInput")
    wqkvT = nc.dram_tensor("wqkvt", [HID, 4 * HD], BF16, kind="ExternalInput")
    woT = nc.dram_tensor("wot", [QH_PER_CORE * HD, HID], BF16, kind="ExternalInput")
    c1d = nc.dram_tensor("c1", [HD, S], F32, kind="ExternalInput")
    c2d = nc.dram_tensor("c2", [HD, S], F32, kind="ExternalInput")
    r1d = nc.dram_tensor("r1t", [HD, HD], BF16, kind="ExternalInput")
    r2d = nc.dram_tensor("r2t", [HD, HD], BF16, kind="ExternalInput")
    outd = nc.dram_tensor("out", [S, HID], F32, kind="ExternalOutput")

    xT_t = xT.rearrange("(ko p) s -> p ko s", p=128)       # [128, 16, 2048]
    wqkv_t = wqkvT.rearrange("(ko p) o -> p ko o", p=128)  # [128, 16, 512]
    wo_t = woT.rearrange("(g p) h -> p g h", p=128)        # [128, 2, 2048]

    with tile.TileContext(nc) as tc, ExitStack() as ctx:
        # ---- pools: one shared PSUM set, exactly 8 banks ------------------
        consts = ctx.enter_context(tc.tile_pool(name="consts", bufs=1))
        persist = ctx.enter_context(tc.tile_pool(name="persist", bufs=1))
        work = ctx.enter_context(tc.tile_pool(name="work", bufs=2))
        expp = ctx.enter_context(tc.tile_pool(name="expp", bufs=4))
        outp = ctx.enter_context(tc.tile_pool(name="outp", bufs=4))
        nrm = ctx.enter_context(tc.tile_pool(name="nrm", bufs=2))

        ps_mm = ctx.enter_context(tc.tile_pool(name="ps_mm", bufs=3, space="PSUM"))
        ps_rv = ctx.enter_context(tc.tile_pool(name="ps_rv", bufs=2, space="PSUM"))
        ps_acc = ctx.enter_context(tc.tile_pool(name="ps_acc", bufs=2, space="PSUM"))
        ps_den = ctx.enter_context(tc.tile_pool(name="ps_den", bufs=1, space="PSUM"))

        # ---- persistent activations ---------------------------------------
        qTs = [persist.tile([128, S], BF16, tag=f"qT{h}", name=f"qT{h}")
               for h in range(QH_PER_CORE)]
        kT = persist.tile([128, S], BF16, tag="kT")
        v_sd = persist.tile([128, NST, HD], BF16, tag="v_sd")
        aoTs = [persist.tile([128, S], BF16, tag=f"aoT{h}", name=f"aoT{h}")
                for h in range(QH_PER_CORE)]

        # ---- input DMAs: first chunk's inputs lead the queues -------------
        wqkv_sb = consts.tile([128, NKC, 512], BF16)
        xt0 = work.tile([128, NKC, SC], BF16, tag="xt", name="xt0")
        for kc in range(NKC):
            eng = nc.sync if kc % 2 == 0 else nc.gpsimd
            eng.dma_start(wqkv_sb[:, kc, :], wqkv_t[:, kc, :])
            eng2 = nc.gpsimd if kc % 2 == 0 else nc.sync
            eng2.dma_start(xt0[:, kc, :], xT_t[:, kc, 0:SC])
        c1_sb = consts.tile([128, S], F32)
        c2_sb = consts.tile([128, S], F32)
        nc.sync.dma_start(c1_sb, c1d[:, :])
        nc.sync.dma_start(c2_sb, c2d[:, :])

        # ---- constants -----------------------------------------------------
        ident = consts.tile([128, 128], BF16)
        make_identity(nc, ident)

        # additive causal mask for the diagonal 128x128 block of a scoresT
        # tile: keep (0) where sq >= sk i.e. col >= row, else MASK_VAL
        trimask = consts.tile([128, 128], F32)
        nc.gpsimd.memset(trimask, 0.0)
        nc.gpsimd.affine_select(
            out=trimask,
            in_=trimask,
            compare_op=mybir.AluOpType.is_ge,
            fill=MASK_VAL,
            base=0,
            pattern=[[1, 128]],       # + 1*col
            channel_multiplier=-1,    # - row
        )

        ones_col = consts.tile([128, 1], BF16)
        nc.vector.memset(ones_col, 1.0)

        r1_sb = consts.tile([128, 128], BF16)
        r2_sb = consts.tile([128, 128], BF16)
        nc.sync.dma_start(r1_sb, r1d[:, :])
        nc.sync.dma_start(r2_sb, r2d[:, :])
        wo_sb = consts.tile([128, QH_PER_CORE, HID], BF16)
        for g in range(QH_PER_CORE):
            for hc in range(NSC):
                nc.sync.dma_start(
                    wo_sb[:, g, hc * SC:(hc + 1) * SC],
                    wo_t[:, g, hc * SC:(hc + 1) * SC],
                )

        def rope_into(dst_chunk, psum_raw, j):
            """dst = C1*(R1@raw) + C2*(R2@raw), raw read from PSUM."""
            raw = work.tile([128, SC], BF16, tag="rope_raw", name="rope_raw")
            nc.vector.tensor_copy(out=raw, in_=psum_raw)
            ps_u = ps_rv.tile([128, SC], F32, tag="rv", name="ps_u")
            ps_w = ps_rv.tile([128, SC], F32, tag="rv", name="ps_w")
            nc.tensor.matmul(ps_u, r1_sb, raw, start=True, stop=True)
            nc.tensor.matmul(ps_w, r2_sb, raw, start=True, stop=True)
            csl = slice(j * SC, (j + 1) * SC)
            t1 = work.tile([128, SC], F32, tag="rope_t1", name="t1")
            t2 = work.tile([128, SC], F32, tag="rope_t2", name="t2")
            nc.vector.tensor_mul(out=t1, in0=ps_u, in1=c1_sb[:, csl])
            nc.vector.tensor_mul(out=t2, in0=ps_w, in1=c2_sb[:, csl])
            nc.vector.tensor_add(out=dst_chunk, in0=t1, in1=t2)

        def proj_chunk(j):
            csl = slice(j * SC, (j + 1) * SC)
            if j == 0:
                xt = xt0
            else:
                xt = work.tile([128, NKC, SC], BF16, tag="xt", name="xt")
                for kc in range(NKC):
                    nc.sync.dma_start(xt[:, kc, :], xT_t[:, kc, csl])

            for t in range(4):  # q0, q1, k, v (columns of wqkv)
                osl = slice(t * 128, (t + 1) * 128)
                ps_p = ps_mm.tile([128, SC], F32, tag="mm", name="ps_p")
                for kc in range(NKC):
                    nc.tensor.matmul(
                        ps_p,
                        wqkv_sb[:, kc, osl],
                        xt[:, kc, :],
                        start=(kc == 0),
                        stop=(kc == NKC - 1),
                    )
                if t < 2:
                    rope_into(qTs[t][:, csl], ps_p, j)
                elif t == 2:
                    rope_into(kT[:, csl], ps_p, j)
                else:
                    # V^T chunk -> bf16 -> PE-transpose into [s,d] tiles
                    vt_raw = work.tile([128, SC], BF16, tag="vt_raw",
                                       name="vt_raw")
                    nc.vector.tensor_copy(out=vt_raw, in_=ps_p)
                    for b in range(SC // 128):
                        st = j * (SC // 128) + b
                        ps_t = ps_rv.tile([128, 128], BF16, tag="rv",
                                          name="ps_t")
                        nc.tensor.transpose(
                            ps_t, vt_raw[:, b * 128:(b + 1) * 128], ident
                        )
                        nc.vector.tensor_copy(out=v_sd[:, st, :], in_=ps_t)

        def attention_chunk(j):
            csl = slice(j * SC, (j + 1) * SC)
            for h in range(QH_PER_CORE):
                qc = qTs[h][:, csl]
                nk = 4 * j + 4  # causal: sk tiles 0..4j+3
                ps_o = ps_acc.tile([128, SC], F32, tag="attn", name="ps_o")
                ps_d = ps_den.tile([1, SC], F32, tag="den", name="ps_d")
                for k in range(nk):
                    m = k - 4 * j
                    # diagonal tiles (m>=0) only touch columns >= 128m; the
                    # masked-out left part is never computed nor read
                    v0 = max(m, 0) * 128
                    vsl = slice(v0, SC)
                    ps_s = ps_mm.tile([128, SC], F32, tag="mm", name="ps_s")
                    nc.tensor.matmul(
                        ps_s[:, vsl], kT[:, k * 128:(k + 1) * 128],
                        qc[:, vsl], start=True, stop=True,
                    )
                    e = expp.tile([128, SC], BF16, tag="exp", name="e")
                    if m >= 0:
                        # triangular mask on the 128-wide diagonal block
                        dsl = slice(m * 128, (m + 1) * 128)
                        nc.vector.tensor_add(
                            out=ps_s[:, dsl], in0=ps_s[:, dsl], in1=trimask
                        )
                    nc.scalar.activation(
                        out=e[:, vsl], in_=ps_s[:, vsl],
                        func=mybir.ActivationFunctionType.Exp,
                        scale=SCALE,
                    )
                    nc.tensor.matmul(
                        ps_o[:, vsl], v_sd[:, k, :], e[:, vsl],
                        start=(k == 0), stop=(k == nk - 1),
                    )
                    nc.tensor.matmul(
                        ps_d[:, vsl], ones_col, e[:, vsl],
                        start=(k == 0), stop=(k == nk - 1),
                    )
                # normalize: attnT[:, sq] *= 1/den[sq]; 1/den = exp(-ln d)
                # on ACT, reading the den bank directly (fast release)
                lnd = nrm.tile([1, SC], F32, tag="lnd", name="lnd")
                nc.scalar.activation(
                    out=lnd, in_=ps_d,
                    func=mybir.ActivationFunctionType.Ln,
                )
                recip = nrm.tile([1, SC], F32, tag="recip", name="recip")
                nc.scalar.activation(
                    out=recip, in_=lnd,
                    func=mybir.ActivationFunctionType.Exp,
                    scale=-1.0,
                )
                db = nrm.tile([128, SC], F32, tag="den_b", name="db")
                nc.gpsimd.partition_broadcast(db, recip[:1, :])
                nc.vector.tensor_mul(out=aoTs[h][:, csl], in0=ps_o, in1=db)

        def outproj_chunk(j):
            for b in range(SC // 128):
                st = j * (SC // 128) + b
                ssl = slice(st * 128, (st + 1) * 128)
                for hc in range(NSC):
                    hsl = slice(hc * SC, (hc + 1) * SC)
                    ps_c = ps_mm.tile([128, SC], F32, tag="mm", name="ps_c")
                    for h in range(QH_PER_CORE):
                        nc.tensor.matmul(
                            ps_c,
                            aoTs[h][:, ssl],
                            wo_sb[:, h, hsl],
                            start=(h == 0),
                            stop=(h == QH_PER_CORE - 1),
                        )
                    ob = outp.tile([128, SC], F32, tag="outb", name="ob")
                    if hc % 2 == 0:
                        nc.vector.tensor_copy(out=ob, in_=ps_c)
                    else:
                        nc.scalar.copy(out=ob, in_=ps_c)
                    eng = nc.sync if (b + hc) % 2 == 0 else nc.gpsimd
                    eng.dma_start(outd[ssl, hsl], ob)

        for j in range(NSC):
            proj_chunk(j)
        attention_chunk(0)
        for j in range(1, NSC):
            attention_chunk(j)
            outproj_chunk(j - 1)
        outproj_chunk(NSC - 1)

    nc.finalize()
    return nc


def shard_inputs(x, cos, sin, wq, wk, wv, wo):
    x = np.asarray(x, np.float32).reshape(S, HID)
    cos = np.asarray(cos, np.float32)
    sin = np.asarray(sin, np.float32)
    wq = np.asarray(wq, np.float32)
    wk = np.asarray(wk, np.float32)
    wv = np.asarray(wv, np.float32)
    wo = np.asarray(wo, np.float32)

    xT = np.ascontiguousarray(x.T).astype(NPBF)

    cos_h, sin_h = cos[:, :HH].T, sin[:, :HH].T       # [64, S]
    c1 = np.ascontiguousarray(np.concatenate([cos_h, -sin_h], axis=0))
    c2 = np.ascontiguousarray(np.concatenate([sin_h, cos_h], axis=0))

    r1 = np.zeros((HD, HD), np.float32)
    for i in range(HH // 2):
        r1[2 * i, 2 * i + 1] = -1.0
        r1[2 * i + 1, 2 * i] = 1.0
    r1[HH:, :] = r1[:HH, :]
    r2 = np.zeros((HD, HD), np.float32)
    for d in range(HH):
        r2[d, d + HH] = 1.0
        r2[d + HH, d + HH] = 1.0
    r1t = np.ascontiguousarray(r1.T).astype(NPBF)  # lhsT for out = R1 @ rhs
    r2t = np.ascontiguousarray(r2.T).astype(NPBF)

    in_maps = []
    for c in range(NCORES):
        h0 = QH_PER_CORE * c
        kvh = h0 * NKV // NH
        wq_c = wq[h0 * HD:(h0 + QH_PER_CORE) * HD, :]    # [256, HID]
        wk_c = wk[kvh * HD:(kvh + 1) * HD, :]            # [128, HID]
        wv_c = wv[kvh * HD:(kvh + 1) * HD, :]
        wqkvT_c = np.ascontiguousarray(
            np.concatenate([wq_c, wk_c, wv_c], axis=0).T).astype(NPBF)
        woT_c = np.ascontiguousarray(
            wo[:, h0 * HD:(h0 + QH_PER_CORE) * HD].T).astype(NPBF)
        in_maps.append({
            "xt": xT,
            "wqkvt": wqkvT_c,
            "wot": woT_c,
            "c1": c1,
            "c2": c2,
            "r1t": r1t,
            "r2t": r2t,
        })
    return in_maps


_CACHED_NC = None


def kernel(x, cos, sin, wq, wk, wv, wo, _trace=False, _tmpdir=None):
    global _CACHED_NC
    in_maps = shard_inputs(x, cos, sin, wq, wk, wv, wo)
    if _CACHED_NC is None:
        _CACHED_NC = build_graph()
    nc = _CACHED_NC
    res = bass_utils.run_bass_kernel_spmd(
        nc, in_maps, core_ids=list(range(NCORES)),
        trace=_trace, tmpdir=_tmpdir,
    )
    total = np.zeros((S, HID), np.float32)
    for r in res.results:
        total += r["out"]
    out = total.reshape(1, S, HID)
    if _trace:
        return out, res
    return out
